# revision 2
# baseline (speedup 1.0000x reference)
"""DeepSeek-V3 MLA forward (B=1, S=2048, D=4096, H=32) on 8 TRN2 NeuronCores.

Sharding: the expensive low-rank a-projections are sharded over SEQUENCE
(each core computes hs@Wa for its 256 positions, 8x less work than
replication), then:
  - compressed kv + shared rope key are AllGathered (576x2048 bf16),
  - q_b is computed locally for ALL 32 heads over the core's 256 positions
    (rms scale + rope folded in), then AllToAll'ed in two head-pair halves
    so attention on the first two heads overlaps the second transfer.
Attention + out-proj are tensor-parallel over heads (4 heads/core); the
post-out-proj all-reduce is done host-side while unsharding (sum of per-core
fp16 partials).

Layout: all activations feature-major (x^T: [feat(part), seq(free)]).
Scores are computed TRANSPOSED (sT[k, q]) so softmax's denominator is a PE
ones-matmul column sum, exp is a plain ACT pass, and P@V consumes expT as
the moving operand. Score/exp/den/PV matmuls are column-sliced on diagonal
blocks (causal masking). RMSNorm scales are deferred: ln weights and
1/sqrt(QD) fold into host-preprocessed weights; the rsqrt row scales are
applied via PE outer-product broadcasts before the collectives.

Scheduling: collective-feeding writes go on the HWDGE queues; collective-
dependent loads go on the gpsimd SWDGE queue (so a blocked load cannot
head-of-line-block the weight streams) and are emitted after their
collective in program order (DRAM dataflow tracking requires it).
"""

import math
from dataclasses import dataclass

import ml_dtypes
import numpy as np

import concourse.bass as bass
import concourse.mybir as mybir
import concourse.tile as tile
from concourse import bacc
from concourse.bass_utils import run_bass_kernel_spmd

F32 = mybir.dt.float32
F32R = mybir.dt.float32r
BF16 = mybir.dt.bfloat16
F16 = mybir.dt.float16
F8E4 = mybir.dt.float8e4
AF = mybir.ActivationFunctionType
BF16NP = ml_dtypes.bfloat16
F8E4NP = ml_dtypes.float8_e4m3fn
WO_SCALE = 32.0

N_CORES = 8
EPS = 1e-6
THETA = 10000.0


@dataclass(frozen=True)
class Cfg:
    S: int = 2048
    D: int = 4096
    QR: int = 1536      # q lora rank
    KVR: int = 512      # kv lora rank
    H: int = 32         # total heads
    HPC: int = 4        # heads per core
    NOPE: int = 128
    ROPE: int = 64
    VD: int = 128

    @property
    def QD(self):
        return self.NOPE + self.ROPE

    @property
    def SC(self):          # per-core seq chunk for the a-projection
        return self.S // N_CORES

    @property
    def DCH(self):
        return self.D // 128

    @property
    def QRCH(self):
        return self.QR // 128

    @property
    def KVCH(self):
        return self.KVR // 128

    @property
    def AM(self):          # a-proj m-chunks: kv + rope(64pad128) + q rank
        return self.KVCH + 1 + self.QRCH

    @property
    def NQT(self):         # 512-wide query tiles
        return self.S // 512

    @property
    def NKI(self):         # 128-wide key blocks
        return self.S // 128

    @property
    def NQB(self):         # q_b output chunks: per group 4 nope + 2 rope
        return N_CORES * 6


FULL = Cfg()


# --------------------------------------------------------------------------
# host-side input preparation
# --------------------------------------------------------------------------

def _rope_perm(rope):
    # deepseek interleave: xp = concat(x[0::2], x[1::2]) acting on rope dims
    return np.concatenate([np.arange(0, rope, 2), np.arange(1, rope, 2)])


def prep_inputs(cfg, hidden_states, Wq_a, q_a_ln_w, Wq_b, Wkv_a, kv_a_ln_w,
                Wkv_b, Wo):
    c = cfg
    hs = np.asarray(hidden_states, np.float32).reshape(c.S, c.D)
    Wq_a = np.asarray(Wq_a, np.float32)
    Wq_b = np.asarray(Wq_b, np.float32)
    Wkv_a = np.asarray(Wkv_a, np.float32)
    Wkv_b = np.asarray(Wkv_b, np.float32)
    Wo = np.asarray(Wo, np.float32)
    q_a_ln_w = np.asarray(q_a_ln_w, np.float32)
    kv_a_ln_w = np.asarray(kv_a_ln_w, np.float32)

    # hidden^T, per-core 256-col slices, pre-swizzled: [128, DCH, SC]
    hT = np.ascontiguousarray(hs.T)                      # [D, S]
    hT = hT.reshape(c.DCH, 128, N_CORES, c.SC)
    hT = np.ascontiguousarray(hT.transpose(2, 1, 0, 3))  # [core, 128, DCH, SC]
    hT = hT.astype(BF16NP)

    # combined a-proj weight, kv chunks FIRST (so the c gather can issue
    # early), then the padded rope chunk, then the q chunks:
    #   wa2 cols = [kv (512) | rope-permuted (64) | pad (64) | qa (1536)]
    perm_a = _rope_perm(c.ROPE)
    wa2 = np.concatenate(
        [Wkv_a[:, :c.KVR], Wkv_a[:, c.KVR:][:, perm_a],
         np.zeros((c.D, 64), np.float32), Wq_a], axis=1)  # [D, AM*128]
    wa = wa2.reshape(c.DCH, 128, c.AM, 128)
    wa = np.ascontiguousarray(wa.transpose(2, 1, 0, 3))   # [AM, 128, DCH, 128]
    wa = wa.astype(BF16NP)

    # full Wq_b (ln + 1/sqrt(QD) folded, rope interleave permuted), grouped
    # into 48 chunks of 128 out-features: group g (dest core) has chunks
    # [nope h=4g..4g+3, rope (4g,4g+1), rope (4g+2,4g+3)].
    qd, nope, rope, vd = c.QD, c.NOPE, c.ROPE, c.VD
    scale = qd ** (-0.5)
    wqb_all = (Wq_b * q_a_ln_w[:, None]).reshape(c.QR, c.H, qd) * scale
    perm = _rope_perm(rope)
    wqb_nope = wqb_all[:, :, :nope]                       # [QR, H, 128]
    wqb_rope = wqb_all[:, :, nope:][:, :, perm]           # [QR, H, 64]
    chunks = []
    for g in range(N_CORES):
        for i in range(6):
            if i < 4:
                col = wqb_nope[:, 4 * g + i]              # [QR, 128]
            else:
                h0 = 4 * g + 2 * (i - 4)
                col = np.concatenate(
                    [wqb_rope[:, h0], wqb_rope[:, h0 + 1]], axis=1)
            # [QR, 128] -> [128, QRCH, 128]: element [p, k, h] = col[128k+p, h]
            chunks.append(col.reshape(c.QRCH, 128, 128).transpose(1, 0, 2))
    wqb = np.ascontiguousarray(np.stack(chunks)).astype(BF16NP)

    wkv_all = (Wkv_b * kv_a_ln_w[:, None]).reshape(c.KVR, c.H, nope + vd)

    # rotary tables, feature-major, replicated to 128 rows; per-core slices
    inv_freq = 1.0 / (THETA ** (np.arange(0, rope, 2, np.float32) / rope))
    freqs = np.outer(np.arange(c.S, dtype=np.float32), inv_freq)  # [S, 32]
    cosT = np.tile(np.cos(freqs).T, (4, 1)).astype(BF16NP)        # [128, S]
    sinT = np.tile(np.sin(freqs).T, (4, 1)).astype(BF16NP)
    # rotate-half as a PE matmul: rot = R @ x with R block-diag over two
    # 64-row rope groups, R = [[0, -I32], [I32, 0]] per group. lhsT = R.T.
    R = np.zeros((128, 128), np.float32)
    for blk in (0, 64):
        for i in range(32):
            R[blk + i, blk + i + 32] = -1.0
            R[blk + i + 32, blk + i] = 1.0
    rotT = np.ascontiguousarray(R.T)

    # diagonal-tile masks: mask01[j][r, q] = 1 if 128*j + r <= q
    j = np.arange(4)[:, None, None]
    r = np.arange(128)[None, :, None]
    q = np.arange(512)[None, None, :]
    mask01 = ((128 * j + r) <= q).astype(BF16NP)

    shared = {
        "wa": wa,
        "wqb": wqb,
        "rotT": rotT,
        "ones_f": np.ones((128, 128), np.float32),
        "mask01": mask01,
    }
    in_maps = []
    for core in range(N_CORES):
        hsel = np.arange(core * c.HPC, (core + 1) * c.HPC)
        wkb_c = np.ascontiguousarray(
            wkv_all[:, hsel, :nope].reshape(c.KVCH, 128, c.HPC * nope)
        ).astype(BF16NP)
        wv_c = np.ascontiguousarray(
            wkv_all[:, hsel, nope:].reshape(c.KVCH, 128, c.HPC * vd)
        ).astype(BF16NP)
        wo_c = np.ascontiguousarray(
            Wo.reshape(c.H, vd, c.D)[hsel]).astype(BF16NP)
        sl = slice(core * c.SC, (core + 1) * c.SC)
        in_maps.append(dict(
            shared, hT=hT[core],
            cosT=np.ascontiguousarray(cosT[:, sl]),
            sinT=np.ascontiguousarray(sinT[:, sl]),
            wkb=wkb_c, wv=wv_c, wo=wo_c))
    return in_maps


# --------------------------------------------------------------------------
# kernel builder
# --------------------------------------------------------------------------

def build(cfg):
    c = cfg
    nc = bacc.Bacc("TRN2", target_bir_lowering=False, debug=False,
                   num_devices=N_CORES)

    hT_d = nc.declare_dram_parameter("hT", [128, c.DCH, c.SC], BF16, isOutput=False)
    wa_d = nc.declare_dram_parameter("wa", [c.AM, 128, c.DCH, 128], BF16, isOutput=False)
    wqb_d = nc.declare_dram_parameter("wqb", [c.NQB, 128, c.QRCH, 128], BF16, isOutput=False)
    wkb_d = nc.declare_dram_parameter("wkb", [c.KVCH, 128, c.HPC * c.NOPE], BF16, isOutput=False)
    wv_d = nc.declare_dram_parameter("wv", [c.KVCH, 128, c.HPC * c.VD], BF16, isOutput=False)
    wo_d = nc.declare_dram_parameter("wo", [c.HPC, 128, c.D], BF16, isOutput=False)
    cos_d = nc.declare_dram_parameter("cosT", [128, c.SC], BF16, isOutput=False)
    sin_d = nc.declare_dram_parameter("sinT", [128, c.SC], BF16, isOutput=False)
    rot_d = nc.declare_dram_parameter("rotT", [128, 128], F32R, isOutput=False)
    ones_d = nc.declare_dram_parameter("ones_f", [128, 128], F32R, isOutput=False)
    mask_d = nc.declare_dram_parameter("mask01", [4, 128, 512], BF16, isOutput=False)
    out_d = nc.declare_dram_parameter("outT", [c.D, c.S], F16, isOutput=True)

    # collective scratch (DRAM)
    CKR = c.KVR + c.ROPE                                   # 576
    c_in = nc.dram_tensor("c_in", [CKR, c.SC], BF16)
    c_all = nc.dram_tensor("c_all", [N_CORES, CKR, c.SC], BF16,
                           addr_space="Shared")
    q_in = [nc.dram_tensor(f"q_in{v}", [N_CORES, 3, 128, c.SC], BF16)
            for v in range(2)]
    q_all = [nc.dram_tensor(f"q_all{v}", [N_CORES, 3, 128, c.SC], BF16)
             for v in range(2)]
    GROUP = [list(range(N_CORES))]

    with tile.TileContext(nc) as tc:
        with tc.tile_pool(name="persist", bufs=1) as pp:
            cos_sb = pp.tile([128, c.SC], BF16, name="cos_sb")
            sin_sb = pp.tile([128, c.SC], BF16, name="sin_sb")
            rot_sb = pp.tile([128, 128], F32R, name="rot_sb")
            ones_sb = pp.tile([128, 128], F32R, name="ones_sb")
            ones_col_f = ones_sb[:, 0:1]
            ones_row_f = ones_sb[0:1, :]
            ones_col_b = pp.tile([128, 1], BF16, name="ones_col_b")
            ones_row_b = pp.tile([1, 128], BF16, name="ones_row_b")
            nc.vector.memset(ones_col_b[:], 1.0)
            nc.vector.memset(ones_row_b[:], 1.0)

            # ---------------- phase B/C residents (hoisted so their SBUF
            # does not alias phase-A working tiles) ----------------------
            pBC_cm = tc.tile_pool(name="pBC", bufs=1)
            pBC = pBC_cm.__enter__()
            knopeT = [pBC.tile([128, c.S], BF16, name=f"knopeT_{m}")
                      for m in range(c.HPC)]
            v_sb = [pBC.tile([128, c.HPC * c.VD], BF16, name=f"v_sb_{ki}")
                    for ki in range(c.NKI)]
            krope2 = [pBC.tile([128, c.S], BF16, name=f"krope2_{par}")
                      for par in range(2)]
            qnopeT = [pBC.tile([128, c.S], BF16, name=f"qnopeT_{h}")
                      for h in range(c.HPC)]
            qropeT = [pBC.tile([128, c.S], BF16, name=f"qropeT_{j}")
                      for j in range(2)]
            c_T = [pBC.tile([128, c.S], BF16, name=f"c_T_{kc}")
                   for kc in range(c.KVCH)]
            wkb_sb = pBC.tile([128, c.KVCH, c.HPC * c.NOPE], BF16, name="wkb")
            wv_sb = pBC.tile([128, c.KVCH, c.HPC * c.VD], BF16, name="wv")

            # ---------------- phase A: a-projections on own seq chunk ----
            pA_cm = tc.tile_pool(name="pA", bufs=1)
            pA = pA_cm.__enter__()
            qa_loc = [pA.tile([128, c.SC], BF16, name=f"qa_{k}")
                      for k in range(c.QRCH)]
            sqb = pA.tile([128, c.SC], F32, name="sqb")
            cossq = pA.tile([128, c.SC], F32, name="cossq")
            sinsq = pA.tile([128, c.SC], F32, name="sinsq")
            with tc.tile_pool(name="pA_h", bufs=1) as pAh, \
                 tc.tile_pool(name="pA_w", bufs=4) as pAw, \
                 tc.tile_pool(name="pA_ev", bufs=4) as pAe, \
                 tc.tile_pool(name="pA_ps", bufs=3, space="PSUM") as psA, \
                 tc.tile_pool(name="pA_ps1", bufs=1, space="PSUM") as psA1:
                # the first compute only needs hT + wa[0] + ones: issue those
                # DMAs first so nothing queues ahead of them
                hT_sb = pAh.tile([128, c.DCH, c.SC], BF16, name="hT_sb")
                nc.sync.dma_start(hT_sb[:, 0:8, :], hT_d.ap()[:, 0:8])

                def load_wa(m):
                    wa_sb = pAw.tile([128, c.DCH, 128], BF16, name="wa_sb")
                    nc.sync.dma_start(wa_sb[:], wa_d.ap()[m])
                    return wa_sb

                wa_pref = load_wa(0)
                for j in range(1, 4):
                    k0, k1 = j * c.DCH // 4, (j + 1) * c.DCH // 4
                    nc.sync.dma_start(hT_sb[:, k0:k1, :], hT_d.ap()[:, k0:k1])
                nc.sync.dma_start(ones_sb[:], ones_d.ap())
                nc.sync.dma_start(cos_sb[:], cos_d.ap())
                nc.sync.dma_start(sin_sb[:], sin_d.ap())
                nc.sync.dma_start(rot_sb[:], rot_d.ap())
                nc.sync.dma_start(
                    wkb_sb[:], wkb_d.ap().rearrange("k p c -> p k c"))
                nc.sync.dma_start(
                    wv_sb[:], wv_d.ap().rearrange("k p c -> p k c"))
                nc.vector.memset(krope2[0][:], 0.0)
                nc.vector.memset(krope2[1][:], 0.0)

                ckv_sb = [pAh.tile([128, c.SC], F32R, name=f"ckv_{m}")
                          for m in range(c.KVCH)]
                ssc = psA1.tile([1, c.SC], F32, name="ssc")
                ssq = psA1.tile([1, c.SC], F32, name="ssq")

                def amm(m):
                    nonlocal wa_pref
                    wa_sb = wa_pref if wa_pref is not None else load_wa(m)
                    wa_pref = None
                    ps = psA.tile([128, c.SC], F32, name="psA")
                    for k in range(c.DCH):
                        nc.tensor.matmul(
                            ps[:], wa_sb[:, k, :], hT_sb[:, k, :],
                            start=(k == 0), stop=(k == c.DCH - 1))
                    return ps

                for m in range(c.KVCH):          # compressed kv chunks
                    ps = amm(m)
                    nc.vector.tensor_copy(ckv_sb[m][:], ps[:])
                    x2 = pAe.tile([128, c.SC], F32R, name="x2")
                    nc.vector.tensor_mul(x2[:], ckv_sb[m][:], ckv_sb[m][:])
                    nc.tensor.matmul(ssc[:], ones_col_f, x2[:],
                                     start=(m == 0), stop=(m == c.KVCH - 1))
                # shared rope key chunk (rows 0:64), rope applied in place
                ps = amm(c.KVCH)
                kr = pAe.tile([64, c.SC], F32R, name="kr")
                nc.vector.tensor_copy(kr[:], ps[0:64, :])
                rps = psA.tile([64, c.SC], F32, name="rot_ps", bufs=1)
                nc.tensor.matmul(rps[:], rot_sb[0:64, 0:64], kr[:])
                rk = pAe.tile([64, c.SC], F32, name="rk")
                nc.vector.tensor_copy(rk[:], rps[:])
                ra = pAe.tile([64, c.SC], F32, name="ra")
                rb = pAe.tile([64, c.SC], F32, name="rb")
                nc.vector.tensor_mul(ra[:], kr[:], cos_sb[0:64, :])
                nc.vector.tensor_mul(rb[:], rk[:], sin_sb[0:64, :])
                krope_sb = pAe.tile([64, c.SC], BF16, name="krope_sb")
                nc.vector.tensor_add(krope_sb[:], ra[:], rb[:])
                nc.sync.dma_start(c_in.ap()[c.KVR:CKR, :], krope_sb[:])

                # finalize 1/rms(c), scale kv chunks, ship + gather
                t = pAe.tile([1, c.SC], F32, name="rms_t")
                nc.vector.tensor_scalar(t[:], ssc[:], 1.0 / c.KVR, EPS,
                                        mybir.AluOpType.mult,
                                        mybir.AluOpType.add)
                st = pAe.tile([1, c.SC], F32, name="rms_st")
                nc.scalar.activation(st[:], t[:], AF.Sqrt)
                rc = pAe.tile([1, c.SC], F32R, name="rms_rc")
                with nc.allow_low_precision(reason="fp32r for PE bcast"):
                    nc.vector.reciprocal(rc[:], st[:])
                bc_ps = psA.tile([128, c.SC], F32, name="bc_ps", bufs=1)
                nc.tensor.matmul(bc_ps[:], ones_row_f, rc[:])
                bc_sb = pAe.tile([128, c.SC], F32, name="bc_sb")
                nc.vector.tensor_copy(bc_sb[:], bc_ps[:])
                for m in range(c.KVCH):
                    cb = pAe.tile([128, c.SC], BF16, name="cb")
                    nc.vector.tensor_mul(cb[:], ckv_sb[m][:], bc_sb[:])
                    nc.sync.dma_start(
                        c_in.ap()[m * 128:(m + 1) * 128, :], cb[:])
                nc.gpsimd.collective_compute(
                    "AllGather", mybir.AluOpType.bypass,
                    replica_groups=GROUP,
                    ins=[c_in.ap().opt()], outs=[c_all.ap().opt()])
                # collective-dependent loads: issued on the gpsimd SWDGE queue
                # (so they cannot head-of-line-block the HWDGE weight streams)
                # and AFTER the collective in program order (the DRAM dataflow
                # tracker links a read to the last preceding write)
                nc.gpsimd.dma_start(
                    krope2[0][0:64, :],
                    c_all.ap()[:, c.KVR:CKR, :].rearrange("s p c -> p s c"))
                nc.gpsimd.dma_start(
                    krope2[1][64:128, :],
                    c_all.ap()[:, c.KVR:CKR, :].rearrange("s p c -> p s c"))
                for kc in range(c.KVCH):
                    nc.gpsimd.dma_start(
                        c_T[kc][:],
                        c_all.ap()[:, kc * 128:(kc + 1) * 128, :]
                        .rearrange("s p c -> p s c"))

                for m in range(c.QRCH):          # q lora chunks
                    ps = amm(c.KVCH + 1 + m)
                    nc.vector.tensor_copy(qa_loc[m][:], ps[:])
                    x2 = pAe.tile([128, c.SC], F32R, name="x2")
                    nc.vector.tensor_mul(x2[:], qa_loc[m][:], qa_loc[m][:])
                    nc.tensor.matmul(ssq[:], ones_col_f, x2[:],
                                     start=(m == 0), stop=(m == c.QRCH - 1))
                # finalize 1/rms(qa) broadcast + rope tables scaled by it
                t = pAe.tile([1, c.SC], F32, name="rms_t")
                nc.vector.tensor_scalar(t[:], ssq[:], 1.0 / c.QR, EPS,
                                        mybir.AluOpType.mult,
                                        mybir.AluOpType.add)
                st = pAe.tile([1, c.SC], F32, name="rms_st")
                nc.scalar.activation(st[:], t[:], AF.Sqrt)
                rc = pAe.tile([1, c.SC], F32R, name="rms_rc")
                with nc.allow_low_precision(reason="fp32r for PE bcast"):
                    nc.vector.reciprocal(rc[:], st[:])
                bc_ps = psA.tile([128, c.SC], F32, name="bc_ps", bufs=1)
                nc.tensor.matmul(bc_ps[:], ones_row_f, rc[:])
                nc.vector.tensor_copy(sqb[:], bc_ps[:])
                nc.vector.tensor_mul(cossq[:], cos_sb[:], sqb[:])
                nc.vector.tensor_mul(sinsq[:], sin_sb[:], sqb[:])

            # ---------------- phase A2: q_b for all heads, 2 AllToAlls ---
            with tc.tile_pool(name="pQ", bufs=1) as pQ, \
                 tc.tile_pool(name="pQ_w", bufs=4) as pQw, \
                 tc.tile_pool(name="pQ_ev", bufs=4) as pQe, \
                 tc.tile_pool(name="pQ_ps", bufs=4, space="PSUM") as psQ:
                at_insts = []
                for half in range(2):
                    types = (0, 1, 4) if half == 0 else (2, 3, 5)
                    stage = pQ.tile([128, N_CORES, 3, c.SC], BF16,
                                    name=f"stage_{half}")
                    for g in range(N_CORES):
                        for ti, i in enumerate(types):
                            ch = 6 * g + i
                            wq = pQw.tile([128, c.QRCH, 128], BF16, name="wq")
                            nc.sync.dma_start(wq[:], wqb_d.ap()[ch])
                            ps = psQ.tile([128, c.SC], F32, name="psQ")
                            for k in range(c.QRCH):
                                nc.tensor.matmul(
                                    ps[:], wq[:, k, :], qa_loc[k][:],
                                    start=(k == 0), stop=(k == c.QRCH - 1))
                            dst = stage[:, g, ti, :]
                            if i < 4:
                                nc.vector.tensor_mul(dst, ps[:], sqb[:])
                            else:
                                ro = pQe.tile([128, c.SC], F32R, name="ro")
                                nc.vector.tensor_copy(ro[:], ps[:])
                                rps = psQ.tile([128, c.SC], F32, name="rotq", bufs=2)
                                nc.tensor.matmul(rps[:], rot_sb[:], ro[:])
                                rk = pQe.tile([128, c.SC], F32, name="rk")
                                nc.vector.tensor_copy(rk[:], rps[:])
                                qa_ = pQe.tile([128, c.SC], F32, name="qa_")
                                qb_ = pQe.tile([128, c.SC], F32, name="qb_")
                                nc.vector.tensor_mul(qa_[:], ro[:], cossq[:])
                                nc.vector.tensor_mul(qb_[:], rk[:], sinsq[:])
                                nc.vector.tensor_add(dst, qa_[:], qb_[:])
                    nc.sync.dma_start(
                        q_in[half].ap().rearrange("g t p c -> p g t c"),
                        stage[:])
                    at_insts.append(nc.gpsimd.collective_compute(
                        "AllToAll", mybir.AluOpType.bypass,
                        replica_groups=GROUP,
                        ins=[q_in[half].ap().opt()],
                        outs=[q_all[half].ap().opt()]))
                # q tile loads AFTER both AllToAlls are dispatched: the CC SEQ
                # slot frees before the transfer, so AT2 queues on the
                # collective track while these sit on the Pool queue
                from concourse.tile_rust import add_dep_helper
                for half in range(2):
                    qls = []
                    for t in range(2):
                        qls.append(nc.gpsimd.dma_start(
                            qnopeT[2 * half + t][:],
                            q_all[half].ap()[:, t]
                            .rearrange("s p c -> p s c")))
                    qls.append(nc.gpsimd.dma_start(
                        qropeT[half][:],
                        q_all[half].ap()[:, 2].rearrange("s p c -> p s c")))
                    for ql in qls:
                        add_dep_helper(
                            ql.ins, at_insts[1].ins, sync=False,
                            reason="q loads ordered after AT2 dispatch")
            pA_cm.__exit__(None, None, None)

            # ---------------- phase B: kv b-projection -------------------
            with tc.tile_pool(name="pB_ps", bufs=3, space="PSUM") as psB:
                for m in range(c.HPC):
                    for n in range(c.S // 512):
                        ps = psB.tile([128, 512], F32, name="psB")
                        for kc in range(c.KVCH):
                            nc.tensor.matmul(
                                ps[:],
                                wkb_sb[:, kc, m * 128:(m + 1) * 128],
                                c_T[kc][:, n * 512:(n + 1) * 512],
                                start=(kc == 0), stop=(kc == c.KVCH - 1))
                        nc.vector.tensor_copy(
                            knopeT[m][:, n * 512:(n + 1) * 512], ps[:])
                for ki in range(c.NKI):
                    ps = psB.tile([128, c.HPC * c.VD], F32, name="psB")
                    for kc in range(c.KVCH):
                        nc.tensor.matmul(
                            ps[:],
                            c_T[kc][:, ki * 128:(ki + 1) * 128],
                            wv_sb[:, kc, :], start=(kc == 0),
                            stop=(kc == c.KVCH - 1))
                    nc.vector.tensor_copy(v_sb[ki][:], ps[:])

            # ---------------- phase C: attention + out-proj --------------
            with tc.tile_pool(name="pC", bufs=1) as pC, \
                 tc.tile_pool(name="pCo", bufs=1) as pCo, \
                 tc.tile_pool(name="pCe", bufs=3) as pCe, \
                 tc.tile_pool(name="pCx", bufs=10) as pCx, \
                 tc.tile_pool(name="pC_mm", bufs=2, space="PSUM") as psM, \
                 tc.tile_pool(name="pC_sT", bufs=3, space="PSUM") as psT, \
                 tc.tile_pool(name="pC_oT", bufs=2, space="PSUM") as psO, \
                 tc.tile_pool(name="pC_den", bufs=1, space="PSUM") as psD:
                wo_sb = []
                for k in range(c.HPC):
                    t = pC.tile([128, c.D], BF16, name=f"wo_{k}")
                    nc.sync.dma_start(t[:], wo_d.ap()[k])
                    wo_sb.append(t)
                mask_sb = []
                for j in range(4):
                    t = pC.tile([128, 512], BF16, name=f"mask_{j}")
                    nc.sync.dma_start(t[:], mask_d.ap()[j])
                    mask_sb.append(t)
                oT_sb = [pC.tile([128, c.S], BF16, name=f"oT_{h}")
                         for h in range(c.HPC)]

                def att(h, qi):
                    q0 = qi * 512
                    nki = 4 * (qi + 1)
                    oT_ps = psO.tile([128, 512], F32, name="psO")
                    den_ps = psD.tile([1, 512], F32, name="psD")
                    for ki in range(nki):
                        j = ki - (nki - 4)
                        z0 = 128 * max(j, 0)   # queries < z0 are fully masked
                        sT_ps = psT.tile([128, 512], F32, name="psT")
                        nc.tensor.matmul(
                            sT_ps[:, z0:],
                            knopeT[h][:, ki * 128:(ki + 1) * 128],
                            qnopeT[h][:, q0 + z0:q0 + 512],
                            start=True, stop=False)
                        nc.tensor.matmul(
                            sT_ps[:, z0:],
                            krope2[h % 2][:, ki * 128:(ki + 1) * 128],
                            qropeT[h // 2][:, q0 + z0:q0 + 512],
                            start=False, stop=True)
                        ex = pCx.tile([128, 512], BF16, name="expT")
                        if z0:
                            nc.vector.memset(ex[:, :z0], 0.0)
                        nc.scalar.activation(ex[:, z0:], sT_ps[:, z0:], AF.Exp)
                        if j >= 0:
                            nc.vector.tensor_mul(
                                ex[:, z0:], ex[:, z0:], mask_sb[j][:, z0:])
                        nc.tensor.matmul(den_ps[:, z0:], ones_col_b[:],
                                         ex[:, z0:],
                                         start=(ki == 0), stop=(ki == nki - 1),
                                         skip_group_check=bool(z0))
                        nc.tensor.matmul(
                            oT_ps[:, z0:],
                            v_sb[ki][:, h * c.VD:(h + 1) * c.VD],
                            ex[:, z0:], start=(ki == 0), stop=(ki == nki - 1),
                            skip_group_check=bool(z0))
                    rec = pCe.tile([1, 512], F32R, name="rec")
                    with nc.allow_low_precision(reason="fp32r for PE bcast"):
                        nc.vector.reciprocal(rec[:], den_ps[:])
                    bc_ps = psM.tile([128, 512], F32, name="psm")
                    nc.tensor.matmul(bc_ps[:], ones_row_f, rec[:])
                    bc_sb = pCe.tile([128, 512], F32, name="bc_sb")
                    nc.vector.tensor_copy(bc_sb[:], bc_ps[:])
                    nc.vector.tensor_mul(
                        oT_sb[h][:, q0:q0 + 512], oT_ps[:], bc_sb[:])

                def outproj(qi):
                    q0 = qi * 512
                    ostage = pCo.tile([128, c.DCH, 512], F16, name="ostage")
                    for m in range(c.DCH):
                        ps = psM.tile([128, 512], F32, name="psm")
                        for k in range(c.HPC):
                            nc.tensor.matmul(
                                ps[:], wo_sb[k][:, m * 128:(m + 1) * 128],
                                oT_sb[k][:, q0:q0 + 512], start=(k == 0),
                                stop=(k == c.HPC - 1))
                        nc.vector.tensor_copy(ostage[:, m, :], ps[:])
                        if m % 4 == 3:
                            m0 = m - 3
                            nc.sync.dma_start(
                                out_d.ap()[m0 * 128:(m + 1) * 128,
                                           q0:q0 + 512]
                                .rearrange("(m p) q -> p m q", p=128),
                                ostage[:, m0:m + 1, :])

                # heads 0-1 while the second AllToAll is in flight, then
                # heads 2-3 interleaved with the out-projection per q tile
                for h in range(2):
                    for qi in range(c.NQT):
                        att(h, qi)
                for qi in range(c.NQT):
                    att(2, qi)
                    att(3, qi)
                    outproj(qi)
            pBC_cm.__exit__(None, None, None)
    nc.compile()
    return nc


# --------------------------------------------------------------------------
# public entry point
# --------------------------------------------------------------------------

_CACHED = {}


def _get_nc(cfg):
    key = cfg
    if key not in _CACHED:
        _CACHED[key] = build(cfg)
    return _CACHED[key]


def kernel(hidden_states, Wq_a, q_a_ln_w, Wq_b, Wkv_a, kv_a_ln_w, Wkv_b, Wo):
    cfg = FULL
    in_maps = prep_inputs(cfg, hidden_states, Wq_a, q_a_ln_w, Wq_b, Wkv_a,
                          kv_a_ln_w, Wkv_b, Wo)
    nc = _get_nc(cfg)
    res = run_bass_kernel_spmd(nc, in_maps, core_ids=list(range(N_CORES)))
    acc = np.zeros((cfg.D, cfg.S), np.float32)
    for r in res.results:
        acc += np.asarray(r["outT"], np.float32)
    return np.ascontiguousarray(acc.T).reshape(1, cfg.S, cfg.D)


# revision 4
# speedup vs baseline: 1.0059x; 1.0059x over previous
"""DeepSeek-V3 MLA forward (B=1, S=2048, D=4096, H=32) on 8 TRN2 NeuronCores.

Sharding: the expensive low-rank a-projections are sharded over SEQUENCE
(each core computes hs@Wa for its 256 positions, 8x less work than
replication), then:
  - compressed kv + shared rope key are AllGathered (576x2048 bf16),
  - q_b is computed locally for ALL 32 heads over the core's 256 positions
    (rms scale + rope folded in), then AllToAll'ed in three groups (head 0
    + its rope, head 1, heads 2-3) so attention on the earliest heads
    overlaps the remaining transfers.
Attention + out-proj are tensor-parallel over heads (4 heads/core); the
post-out-proj all-reduce is done host-side while unsharding (sum of per-core
fp16 partials).

Layout: all activations feature-major (x^T: [feat(part), seq(free)]).
Scores are computed TRANSPOSED (sT[k, q]) so softmax's denominator is a PE
ones-matmul column sum, exp is a plain ACT pass, and P@V consumes expT as
the moving operand. Score/exp/den/PV matmuls are column-sliced on diagonal
blocks (causal masking). RMSNorm scales are deferred: ln weights and
1/sqrt(QD) fold into host-preprocessed weights; the rsqrt row scales are
applied via PE outer-product broadcasts before the collectives.

Scheduling: collective-feeding writes go on the HWDGE queues; collective-
dependent loads go on the gpsimd SWDGE queue (so a blocked load cannot
head-of-line-block the weight streams) and are emitted after their
collective in program order (DRAM dataflow tracking requires it).
"""

import math
from dataclasses import dataclass

import ml_dtypes
import numpy as np

import concourse.bass as bass
import concourse.mybir as mybir
import concourse.tile as tile
from concourse import bacc
from concourse.bass_utils import run_bass_kernel_spmd

F32 = mybir.dt.float32
F32R = mybir.dt.float32r
BF16 = mybir.dt.bfloat16
F16 = mybir.dt.float16
F8E4 = mybir.dt.float8e4
AF = mybir.ActivationFunctionType
BF16NP = ml_dtypes.bfloat16
F8E4NP = ml_dtypes.float8_e4m3fn
WO_SCALE = 32.0

N_CORES = 8
EPS = 1e-6
THETA = 10000.0


@dataclass(frozen=True)
class Cfg:
    S: int = 2048
    D: int = 4096
    QR: int = 1536      # q lora rank
    KVR: int = 512      # kv lora rank
    H: int = 32         # total heads
    HPC: int = 4        # heads per core
    NOPE: int = 128
    ROPE: int = 64
    VD: int = 128

    @property
    def QD(self):
        return self.NOPE + self.ROPE

    @property
    def SC(self):          # per-core seq chunk for the a-projection
        return self.S // N_CORES

    @property
    def DCH(self):
        return self.D // 128

    @property
    def QRCH(self):
        return self.QR // 128

    @property
    def KVCH(self):
        return self.KVR // 128

    @property
    def AM(self):          # a-proj m-chunks: kv + rope(64pad128) + q rank
        return self.KVCH + 1 + self.QRCH

    @property
    def NQT(self):         # 512-wide query tiles
        return self.S // 512

    @property
    def NKI(self):         # 128-wide key blocks
        return self.S // 128

    @property
    def NQB(self):         # q_b output chunks: per group 4 nope + 2 rope
        return N_CORES * 6


FULL = Cfg()


# --------------------------------------------------------------------------
# host-side input preparation
# --------------------------------------------------------------------------

def _rope_perm(rope):
    # deepseek interleave: xp = concat(x[0::2], x[1::2]) acting on rope dims
    return np.concatenate([np.arange(0, rope, 2), np.arange(1, rope, 2)])


def prep_inputs(cfg, hidden_states, Wq_a, q_a_ln_w, Wq_b, Wkv_a, kv_a_ln_w,
                Wkv_b, Wo):
    c = cfg
    hs = np.asarray(hidden_states, np.float32).reshape(c.S, c.D)
    Wq_a = np.asarray(Wq_a, np.float32)
    Wq_b = np.asarray(Wq_b, np.float32)
    Wkv_a = np.asarray(Wkv_a, np.float32)
    Wkv_b = np.asarray(Wkv_b, np.float32)
    Wo = np.asarray(Wo, np.float32)
    q_a_ln_w = np.asarray(q_a_ln_w, np.float32)
    kv_a_ln_w = np.asarray(kv_a_ln_w, np.float32)

    # hidden^T, per-core 256-col slices, pre-swizzled: [128, DCH, SC]
    hT = np.ascontiguousarray(hs.T)                      # [D, S]
    hT = hT.reshape(c.DCH, 128, N_CORES, c.SC)
    hT = np.ascontiguousarray(hT.transpose(2, 1, 0, 3))  # [core, 128, DCH, SC]
    hT = hT.astype(BF16NP)

    # combined a-proj weight, kv chunks FIRST (so the c gather can issue
    # early), then the padded rope chunk, then the q chunks:
    #   wa2 cols = [kv (512) | rope-permuted (64) | pad (64) | qa (1536)]
    perm_a = _rope_perm(c.ROPE)
    wa2 = np.concatenate(
        [Wkv_a[:, :c.KVR], Wkv_a[:, c.KVR:][:, perm_a],
         np.zeros((c.D, 64), np.float32), Wq_a], axis=1)  # [D, AM*128]
    wa = wa2.reshape(c.DCH, 128, c.AM, 128)
    wa = np.ascontiguousarray(wa.transpose(2, 1, 0, 3))   # [AM, 128, DCH, 128]
    wa = wa.astype(BF16NP)

    # full Wq_b (ln + 1/sqrt(QD) folded, rope interleave permuted), grouped
    # into 48 chunks of 128 out-features: group g (dest core) has chunks
    # [nope h=4g..4g+3, rope (4g,4g+1), rope (4g+2,4g+3)].
    qd, nope, rope, vd = c.QD, c.NOPE, c.ROPE, c.VD
    scale = qd ** (-0.5)
    wqb_all = (Wq_b * q_a_ln_w[:, None]).reshape(c.QR, c.H, qd) * scale
    perm = _rope_perm(rope)
    wqb_nope = wqb_all[:, :, :nope]                       # [QR, H, 128]
    wqb_rope = wqb_all[:, :, nope:][:, :, perm]           # [QR, H, 64]
    chunks = []
    for g in range(N_CORES):
        for i in range(6):
            if i < 4:
                col = wqb_nope[:, 4 * g + i]              # [QR, 128]
            else:
                h0 = 4 * g + 2 * (i - 4)
                col = np.concatenate(
                    [wqb_rope[:, h0], wqb_rope[:, h0 + 1]], axis=1)
            # [QR, 128] -> [128, QRCH, 128]: element [p, k, h] = col[128k+p, h]
            chunks.append(col.reshape(c.QRCH, 128, 128).transpose(1, 0, 2))
    wqb = np.ascontiguousarray(np.stack(chunks)).astype(BF16NP)

    wkv_all = (Wkv_b * kv_a_ln_w[:, None]).reshape(c.KVR, c.H, nope + vd)

    # rotary tables, feature-major, replicated to 128 rows; per-core slices
    inv_freq = 1.0 / (THETA ** (np.arange(0, rope, 2, np.float32) / rope))
    freqs = np.outer(np.arange(c.S, dtype=np.float32), inv_freq)  # [S, 32]
    cosT = np.tile(np.cos(freqs).T, (4, 1)).astype(BF16NP)        # [128, S]
    sinT = np.tile(np.sin(freqs).T, (4, 1)).astype(BF16NP)
    # rotate-half as a PE matmul: rot = R @ x with R block-diag over two
    # 64-row rope groups, R = [[0, -I32], [I32, 0]] per group. lhsT = R.T.
    R = np.zeros((128, 128), np.float32)
    for blk in (0, 64):
        for i in range(32):
            R[blk + i, blk + i + 32] = -1.0
            R[blk + i + 32, blk + i] = 1.0
    rotT = np.ascontiguousarray(R.T)

    # diagonal-tile masks: mask01[j][r, q] = 1 if 128*j + r <= q
    j = np.arange(4)[:, None, None]
    r = np.arange(128)[None, :, None]
    q = np.arange(512)[None, None, :]
    mask01 = ((128 * j + r) <= q).astype(BF16NP)

    shared = {
        "wa": wa,
        "wqb": wqb,
        "rotT": rotT,
        "ones_f": np.ones((128, 128), np.float32),
        "mask01": mask01,
    }
    in_maps = []
    for core in range(N_CORES):
        hsel = np.arange(core * c.HPC, (core + 1) * c.HPC)
        wkb_c = np.ascontiguousarray(
            wkv_all[:, hsel, :nope].reshape(c.KVCH, 128, c.HPC * nope)
        ).astype(BF16NP)
        wv_c = np.ascontiguousarray(
            wkv_all[:, hsel, nope:].reshape(c.KVCH, 128, c.HPC * vd)
        ).astype(BF16NP)
        wo_c = np.ascontiguousarray(
            Wo.reshape(c.H, vd, c.D)[hsel]).astype(BF16NP)
        sl = slice(core * c.SC, (core + 1) * c.SC)
        in_maps.append(dict(
            shared, hT=hT[core],
            cosT=np.ascontiguousarray(cosT[:, sl]),
            sinT=np.ascontiguousarray(sinT[:, sl]),
            wkb=wkb_c, wv=wv_c, wo=wo_c))
    return in_maps


# --------------------------------------------------------------------------
# kernel builder
# --------------------------------------------------------------------------

def build(cfg):
    c = cfg
    nc = bacc.Bacc("TRN2", target_bir_lowering=False, debug=False,
                   num_devices=N_CORES)

    hT_d = nc.declare_dram_parameter("hT", [128, c.DCH, c.SC], BF16, isOutput=False)
    wa_d = nc.declare_dram_parameter("wa", [c.AM, 128, c.DCH, 128], BF16, isOutput=False)
    wqb_d = nc.declare_dram_parameter("wqb", [c.NQB, 128, c.QRCH, 128], BF16, isOutput=False)
    wkb_d = nc.declare_dram_parameter("wkb", [c.KVCH, 128, c.HPC * c.NOPE], BF16, isOutput=False)
    wv_d = nc.declare_dram_parameter("wv", [c.KVCH, 128, c.HPC * c.VD], BF16, isOutput=False)
    wo_d = nc.declare_dram_parameter("wo", [c.HPC, 128, c.D], BF16, isOutput=False)
    cos_d = nc.declare_dram_parameter("cosT", [128, c.SC], BF16, isOutput=False)
    sin_d = nc.declare_dram_parameter("sinT", [128, c.SC], BF16, isOutput=False)
    rot_d = nc.declare_dram_parameter("rotT", [128, 128], F32R, isOutput=False)
    ones_d = nc.declare_dram_parameter("ones_f", [128, 128], F32R, isOutput=False)
    mask_d = nc.declare_dram_parameter("mask01", [4, 128, 512], BF16, isOutput=False)
    out_d = nc.declare_dram_parameter("outT", [c.D, c.S], F16, isOutput=True)

    # collective scratch (DRAM)
    CKR = c.KVR + c.ROPE                                   # 576
    c_in = nc.dram_tensor("c_in", [CKR, c.SC], BF16)
    c_all = nc.dram_tensor("c_all", [N_CORES, CKR, c.SC], BF16,
                           addr_space="Shared")
    AT_TYPES = ((0, 4), (1,), (2, 3, 5))   # qb chunk types per AllToAll
    q_in = [nc.dram_tensor(f"q_in{v}", [N_CORES, len(ts_), 128, c.SC], BF16)
            for v, ts_ in enumerate(AT_TYPES)]
    q_all = [nc.dram_tensor(f"q_all{v}", [N_CORES, len(ts_), 128, c.SC], BF16)
             for v, ts_ in enumerate(AT_TYPES)]
    GROUP = [list(range(N_CORES))]

    with tile.TileContext(nc) as tc:
        with tc.tile_pool(name="persist", bufs=1) as pp:
            cos_sb = pp.tile([128, c.SC], BF16, name="cos_sb")
            sin_sb = pp.tile([128, c.SC], BF16, name="sin_sb")
            rot_sb = pp.tile([128, 128], F32R, name="rot_sb")
            ones_sb = pp.tile([128, 128], F32R, name="ones_sb")
            ones_col_f = ones_sb[:, 0:1]
            ones_row_f = ones_sb[0:1, :]
            ones_col_b = pp.tile([128, 1], BF16, name="ones_col_b")
            ones_row_b = pp.tile([1, 128], BF16, name="ones_row_b")
            nc.vector.memset(ones_col_b[:], 1.0)
            nc.vector.memset(ones_row_b[:], 1.0)

            # ---------------- phase B/C residents (hoisted so their SBUF
            # does not alias phase-A working tiles) ----------------------
            pBC_cm = tc.tile_pool(name="pBC", bufs=1)
            pBC = pBC_cm.__enter__()
            knopeT = [pBC.tile([128, c.S], BF16, name=f"knopeT_{m}")
                      for m in range(c.HPC)]
            v_sb = [pBC.tile([128, c.HPC * c.VD], BF16, name=f"v_sb_{ki}")
                    for ki in range(c.NKI)]
            krope2 = [pBC.tile([128, c.S], BF16, name=f"krope2_{par}")
                      for par in range(2)]
            qnopeT = [pBC.tile([128, c.S], BF16, name=f"qnopeT_{h}")
                      for h in range(c.HPC)]
            qropeT = [pBC.tile([128, c.S], BF16, name=f"qropeT_{j}")
                      for j in range(2)]
            c_T = [pBC.tile([128, c.S], BF16, name=f"c_T_{kc}")
                   for kc in range(c.KVCH)]
            wkb_sb = pBC.tile([128, c.KVCH, c.HPC * c.NOPE], BF16, name="wkb")
            wv_sb = pBC.tile([128, c.KVCH, c.HPC * c.VD], BF16, name="wv")

            # ---------------- phase A: a-projections on own seq chunk ----
            pA_cm = tc.tile_pool(name="pA", bufs=1)
            pA = pA_cm.__enter__()
            qa_loc = [pA.tile([128, c.SC], BF16, name=f"qa_{k}")
                      for k in range(c.QRCH)]
            sqb = pA.tile([128, c.SC], F32, name="sqb")
            cossq = pA.tile([128, c.SC], F32, name="cossq")
            sinsq = pA.tile([128, c.SC], F32, name="sinsq")
            with tc.tile_pool(name="pA_h", bufs=1) as pAh, \
                 tc.tile_pool(name="pA_w", bufs=5) as pAw, \
                 tc.tile_pool(name="pA_ev", bufs=4) as pAe, \
                 tc.tile_pool(name="pA_ps", bufs=3, space="PSUM") as psA, \
                 tc.tile_pool(name="pA_ps1", bufs=1, space="PSUM") as psA1:
                # the first compute only needs hT + wa[0] + ones: issue those
                # DMAs first so nothing queues ahead of them
                hT_sb = pAh.tile([128, c.DCH, c.SC], BF16, name="hT_sb")
                nc.sync.dma_start(hT_sb[:, 0:8, :], hT_d.ap()[:, 0:8])

                def load_wa(m):
                    wa_sb = pAw.tile([128, c.DCH, 128], BF16, name="wa_sb")
                    nc.sync.dma_start(wa_sb[:], wa_d.ap()[m])
                    return wa_sb

                wa_pref = load_wa(0)
                for j in range(1, 4):
                    k0, k1 = j * c.DCH // 4, (j + 1) * c.DCH // 4
                    nc.sync.dma_start(hT_sb[:, k0:k1, :], hT_d.ap()[:, k0:k1])
                nc.sync.dma_start(ones_sb[:], ones_d.ap())
                nc.sync.dma_start(cos_sb[:], cos_d.ap())
                nc.sync.dma_start(sin_sb[:], sin_d.ap())
                nc.sync.dma_start(rot_sb[:], rot_d.ap())
                nc.sync.dma_start(
                    wkb_sb[:], wkb_d.ap().rearrange("k p c -> p k c"))
                nc.sync.dma_start(
                    wv_sb[:], wv_d.ap().rearrange("k p c -> p k c"))
                nc.vector.memset(krope2[0][:], 0.0)
                nc.vector.memset(krope2[1][:], 0.0)

                ckv_sb = [pAh.tile([128, c.SC], F32R, name=f"ckv_{m}")
                          for m in range(c.KVCH)]
                ssc = psA1.tile([1, c.SC], F32, name="ssc")
                ssq = psA1.tile([1, c.SC], F32, name="ssq")

                def amm(m):
                    nonlocal wa_pref
                    wa_sb = wa_pref if wa_pref is not None else load_wa(m)
                    wa_pref = None
                    ps = psA.tile([128, c.SC], F32, name="psA")
                    for k in range(c.DCH):
                        nc.tensor.matmul(
                            ps[:], wa_sb[:, k, :], hT_sb[:, k, :],
                            start=(k == 0), stop=(k == c.DCH - 1))
                    return ps

                for m in range(c.KVCH):          # compressed kv chunks
                    ps = amm(m)
                    nc.vector.tensor_copy(ckv_sb[m][:], ps[:])
                    x2 = pAe.tile([128, c.SC], F32R, name="x2")
                    nc.vector.tensor_mul(x2[:], ckv_sb[m][:], ckv_sb[m][:])
                    nc.tensor.matmul(ssc[:], ones_col_f, x2[:],
                                     start=(m == 0), stop=(m == c.KVCH - 1))
                # shared rope key chunk (rows 0:64), rope applied in place
                ps = amm(c.KVCH)
                kr = pAe.tile([64, c.SC], F32R, name="kr")
                nc.vector.tensor_copy(kr[:], ps[0:64, :])
                rps = psA.tile([64, c.SC], F32, name="rot_ps", bufs=1)
                nc.tensor.matmul(rps[:], rot_sb[0:64, 0:64], kr[:])
                rk = pAe.tile([64, c.SC], F32, name="rk")
                nc.vector.tensor_copy(rk[:], rps[:])
                ra = pAe.tile([64, c.SC], F32, name="ra")
                rb = pAe.tile([64, c.SC], F32, name="rb")
                nc.vector.tensor_mul(ra[:], kr[:], cos_sb[0:64, :])
                nc.vector.tensor_mul(rb[:], rk[:], sin_sb[0:64, :])
                krope_sb = pAe.tile([64, c.SC], BF16, name="krope_sb")
                nc.vector.tensor_add(krope_sb[:], ra[:], rb[:])
                nc.sync.dma_start(c_in.ap()[c.KVR:CKR, :], krope_sb[:])

                # finalize 1/rms(c), scale kv chunks, ship + gather
                t = pAe.tile([1, c.SC], F32, name="rms_t")
                nc.vector.tensor_scalar(t[:], ssc[:], 1.0 / c.KVR, EPS,
                                        mybir.AluOpType.mult,
                                        mybir.AluOpType.add)
                st = pAe.tile([1, c.SC], F32, name="rms_st")
                nc.scalar.activation(st[:], t[:], AF.Sqrt)
                rc = pAe.tile([1, c.SC], F32R, name="rms_rc")
                with nc.allow_low_precision(reason="fp32r for PE bcast"):
                    nc.vector.reciprocal(rc[:], st[:])
                bc_ps = psA.tile([128, c.SC], F32, name="bc_ps", bufs=1)
                nc.tensor.matmul(bc_ps[:], ones_row_f, rc[:])
                bc_sb = pAe.tile([128, c.SC], F32, name="bc_sb")
                nc.vector.tensor_copy(bc_sb[:], bc_ps[:])
                for m in range(c.KVCH):
                    cb = pAe.tile([128, c.SC], BF16, name="cb")
                    nc.vector.tensor_mul(cb[:], ckv_sb[m][:], bc_sb[:])
                    nc.sync.dma_start(
                        c_in.ap()[m * 128:(m + 1) * 128, :], cb[:])
                nc.gpsimd.collective_compute(
                    "AllGather", mybir.AluOpType.bypass,
                    replica_groups=GROUP,
                    ins=[c_in.ap().opt()], outs=[c_all.ap().opt()])
                # collective-dependent loads: issued on the gpsimd SWDGE queue
                # (so they cannot head-of-line-block the HWDGE weight streams)
                # and AFTER the collective in program order (the DRAM dataflow
                # tracker links a read to the last preceding write)
                nc.gpsimd.dma_start(
                    krope2[0][0:64, :],
                    c_all.ap()[:, c.KVR:CKR, :].rearrange("s p c -> p s c"))
                nc.gpsimd.dma_start(
                    krope2[1][64:128, :],
                    c_all.ap()[:, c.KVR:CKR, :].rearrange("s p c -> p s c"))
                for kc in range(c.KVCH):
                    nc.gpsimd.dma_start(
                        c_T[kc][:],
                        c_all.ap()[:, kc * 128:(kc + 1) * 128, :]
                        .rearrange("s p c -> p s c"))

                for m in range(c.QRCH):          # q lora chunks
                    ps = amm(c.KVCH + 1 + m)
                    nc.vector.tensor_copy(qa_loc[m][:], ps[:])
                    x2 = pAe.tile([128, c.SC], F32R, name="x2")
                    nc.vector.tensor_mul(x2[:], qa_loc[m][:], qa_loc[m][:])
                    nc.tensor.matmul(ssq[:], ones_col_f, x2[:],
                                     start=(m == 0), stop=(m == c.QRCH - 1))
                # finalize 1/rms(qa) broadcast + rope tables scaled by it
                t = pAe.tile([1, c.SC], F32, name="rms_t")
                nc.vector.tensor_scalar(t[:], ssq[:], 1.0 / c.QR, EPS,
                                        mybir.AluOpType.mult,
                                        mybir.AluOpType.add)
                st = pAe.tile([1, c.SC], F32, name="rms_st")
                nc.scalar.activation(st[:], t[:], AF.Sqrt)
                rc = pAe.tile([1, c.SC], F32R, name="rms_rc")
                with nc.allow_low_precision(reason="fp32r for PE bcast"):
                    nc.vector.reciprocal(rc[:], st[:])
                bc_ps = psA.tile([128, c.SC], F32, name="bc_ps", bufs=1)
                nc.tensor.matmul(bc_ps[:], ones_row_f, rc[:])
                nc.vector.tensor_copy(sqb[:], bc_ps[:])
                nc.vector.tensor_mul(cossq[:], cos_sb[:], sqb[:])
                nc.vector.tensor_mul(sinsq[:], sin_sb[:], sqb[:])

            # ---------------- phase A2: q_b for all heads, 2 AllToAlls ---
            with tc.tile_pool(name="pQ", bufs=1) as pQ, \
                 tc.tile_pool(name="pQ_w", bufs=6) as pQw, \
                 tc.tile_pool(name="pQ_ev", bufs=4) as pQe, \
                 tc.tile_pool(name="pQ_ps", bufs=4, space="PSUM") as psQ:
                at_insts = []
                for half, types in enumerate(AT_TYPES):
                    stage = pQ.tile([128, N_CORES, len(types), c.SC], BF16,
                                    name=f"stage_{half}")
                    for g in range(N_CORES):
                        for ti, i in enumerate(types):
                            ch = 6 * g + i
                            wq = pQw.tile([128, c.QRCH, 128], BF16, name="wq")
                            nc.sync.dma_start(wq[:], wqb_d.ap()[ch])
                            ps = psQ.tile([128, c.SC], F32, name="psQ")
                            for k in range(c.QRCH):
                                nc.tensor.matmul(
                                    ps[:], wq[:, k, :], qa_loc[k][:],
                                    start=(k == 0), stop=(k == c.QRCH - 1))
                            dst = stage[:, g, ti, :]
                            if i < 4:
                                nc.vector.tensor_mul(dst, ps[:], sqb[:])
                            else:
                                ro = pQe.tile([128, c.SC], F32R, name="ro")
                                nc.vector.tensor_copy(ro[:], ps[:])
                                rps = psQ.tile([128, c.SC], F32, name="rotq", bufs=2)
                                nc.tensor.matmul(rps[:], rot_sb[:], ro[:])
                                rk = pQe.tile([128, c.SC], F32, name="rk")
                                nc.vector.tensor_copy(rk[:], rps[:])
                                qa_ = pQe.tile([128, c.SC], F32, name="qa_")
                                qb_ = pQe.tile([128, c.SC], F32, name="qb_")
                                nc.vector.tensor_mul(qa_[:], ro[:], cossq[:])
                                nc.vector.tensor_mul(qb_[:], rk[:], sinsq[:])
                                nc.vector.tensor_add(dst, qa_[:], qb_[:])
                    nc.sync.dma_start(
                        q_in[half].ap().rearrange("g t p c -> p g t c"),
                        stage[:])
                    at_insts.append(nc.gpsimd.collective_compute(
                        "AllToAll", mybir.AluOpType.bypass,
                        replica_groups=GROUP,
                        ins=[q_in[half].ap().opt()],
                        outs=[q_all[half].ap().opt()]))
                # q tile loads AFTER all AllToAlls are dispatched: the CC
                # SEQ slot frees before the transfer, so later ATs queue on
                # the collective track while these sit on the Pool queue.
                # group 0 carries head0 nope + rope01, group 1 head1 nope,
                # group 2 heads 2-3 nope + rope23.
                from concourse.tile_rust import add_dep_helper
                qls = []
                qls.append(nc.gpsimd.dma_start(
                    qnopeT[0][:],
                    q_all[0].ap()[:, 0].rearrange("s p c -> p s c")))
                qls.append(nc.gpsimd.dma_start(
                    qropeT[0][:],
                    q_all[0].ap()[:, 1].rearrange("s p c -> p s c")))
                qls.append(nc.gpsimd.dma_start(
                    qnopeT[1][:],
                    q_all[1].ap()[:, 0].rearrange("s p c -> p s c")))
                for t in range(2):
                    qls.append(nc.gpsimd.dma_start(
                        qnopeT[2 + t][:],
                        q_all[2].ap()[:, t].rearrange("s p c -> p s c")))
                qls.append(nc.gpsimd.dma_start(
                    qropeT[1][:],
                    q_all[2].ap()[:, 2].rearrange("s p c -> p s c")))
                for ql in qls:
                    add_dep_helper(
                        ql.ins, at_insts[2].ins, sync=False,
                        reason="q loads ordered after last AT dispatch")
            pA_cm.__exit__(None, None, None)

            # ---------------- phase B: kv b-projection -------------------
            with tc.tile_pool(name="pB_ps", bufs=3, space="PSUM") as psB:
                for m in range(c.HPC):
                    for n in range(c.S // 512):
                        ps = psB.tile([128, 512], F32, name="psB")
                        for kc in range(c.KVCH):
                            nc.tensor.matmul(
                                ps[:],
                                wkb_sb[:, kc, m * 128:(m + 1) * 128],
                                c_T[kc][:, n * 512:(n + 1) * 512],
                                start=(kc == 0), stop=(kc == c.KVCH - 1))
                        nc.vector.tensor_copy(
                            knopeT[m][:, n * 512:(n + 1) * 512], ps[:])
                for ki in range(c.NKI):
                    ps = psB.tile([128, c.HPC * c.VD], F32, name="psB")
                    for kc in range(c.KVCH):
                        nc.tensor.matmul(
                            ps[:],
                            c_T[kc][:, ki * 128:(ki + 1) * 128],
                            wv_sb[:, kc, :], start=(kc == 0),
                            stop=(kc == c.KVCH - 1))
                    nc.vector.tensor_copy(v_sb[ki][:], ps[:])

            # ---------------- phase C: attention + out-proj --------------
            with tc.tile_pool(name="pC", bufs=1) as pC, \
                 tc.tile_pool(name="pCo", bufs=1) as pCo, \
                 tc.tile_pool(name="pCe", bufs=3) as pCe, \
                 tc.tile_pool(name="pCx", bufs=10) as pCx, \
                 tc.tile_pool(name="pC_mm", bufs=2, space="PSUM") as psM, \
                 tc.tile_pool(name="pC_sT", bufs=3, space="PSUM") as psT, \
                 tc.tile_pool(name="pC_oT", bufs=2, space="PSUM") as psO, \
                 tc.tile_pool(name="pC_den", bufs=1, space="PSUM") as psD:
                wo_sb = []
                for k in range(c.HPC):
                    t = pC.tile([128, c.D], BF16, name=f"wo_{k}")
                    nc.sync.dma_start(t[:], wo_d.ap()[k])
                    wo_sb.append(t)
                mask_sb = []
                for j in range(4):
                    t = pC.tile([128, 512], BF16, name=f"mask_{j}")
                    nc.sync.dma_start(t[:], mask_d.ap()[j])
                    mask_sb.append(t)
                oT_sb = [pC.tile([128, c.S], BF16, name=f"oT_{h}")
                         for h in range(c.HPC)]

                def att(h, qi):
                    q0 = qi * 512
                    nki = 4 * (qi + 1)
                    oT_ps = psO.tile([128, 512], F32, name="psO")
                    den_ps = psD.tile([1, 512], F32, name="psD")
                    for ki in range(nki):
                        j = ki - (nki - 4)
                        z0 = 128 * max(j, 0)   # queries < z0 are fully masked
                        sT_ps = psT.tile([128, 512], F32, name="psT")
                        nc.tensor.matmul(
                            sT_ps[:, z0:],
                            knopeT[h][:, ki * 128:(ki + 1) * 128],
                            qnopeT[h][:, q0 + z0:q0 + 512],
                            start=True, stop=False)
                        nc.tensor.matmul(
                            sT_ps[:, z0:],
                            krope2[h % 2][:, ki * 128:(ki + 1) * 128],
                            qropeT[h // 2][:, q0 + z0:q0 + 512],
                            start=False, stop=True)
                        ex = pCx.tile([128, 512], BF16, name="expT")
                        if z0:
                            nc.vector.memset(ex[:, :z0], 0.0)
                        nc.scalar.activation(ex[:, z0:], sT_ps[:, z0:], AF.Exp)
                        if j >= 0:
                            nc.vector.tensor_mul(
                                ex[:, z0:], ex[:, z0:], mask_sb[j][:, z0:])
                        nc.tensor.matmul(den_ps[:, z0:], ones_col_b[:],
                                         ex[:, z0:],
                                         start=(ki == 0), stop=(ki == nki - 1),
                                         skip_group_check=bool(z0))
                        nc.tensor.matmul(
                            oT_ps[:, z0:],
                            v_sb[ki][:, h * c.VD:(h + 1) * c.VD],
                            ex[:, z0:], start=(ki == 0), stop=(ki == nki - 1),
                            skip_group_check=bool(z0))
                    rec = pCe.tile([1, 512], F32R, name="rec")
                    with nc.allow_low_precision(reason="fp32r for PE bcast"):
                        nc.vector.reciprocal(rec[:], den_ps[:])
                    bc_ps = psM.tile([128, 512], F32, name="psm")
                    nc.tensor.matmul(bc_ps[:], ones_row_f, rec[:])
                    bc_sb = pCe.tile([128, 512], F32, name="bc_sb")
                    nc.vector.tensor_copy(bc_sb[:], bc_ps[:])
                    nc.vector.tensor_mul(
                        oT_sb[h][:, q0:q0 + 512], oT_ps[:], bc_sb[:])

                def outproj(qi):
                    q0 = qi * 512
                    ostage = pCo.tile([128, c.DCH, 512], F16, name="ostage")
                    for m in range(c.DCH):
                        ps = psM.tile([128, 512], F32, name="psm")
                        for k in range(c.HPC):
                            nc.tensor.matmul(
                                ps[:], wo_sb[k][:, m * 128:(m + 1) * 128],
                                oT_sb[k][:, q0:q0 + 512], start=(k == 0),
                                stop=(k == c.HPC - 1))
                        nc.vector.tensor_copy(ostage[:, m, :], ps[:])
                        step = 2 if (qi == c.NQT - 1 and m >= 24) else 4
                        if m % step == step - 1:
                            m0 = m - step + 1
                            nc.sync.dma_start(
                                out_d.ap()[m0 * 128:(m + 1) * 128,
                                           q0:q0 + 512]
                                .rearrange("(m p) q -> p m q", p=128),
                                ostage[:, m0:m + 1, :])

                # head 0 while AT-b/AT-c are in flight, head 1 while AT-c
                # is in flight, then heads 2-3 interleaved with the
                # out-projection per q tile
                for qi in range(c.NQT):
                    att(0, qi)
                for qi in range(c.NQT):
                    att(1, qi)
                for qi in range(c.NQT):
                    att(2, qi)
                    att(3, qi)
                    outproj(qi)
            pBC_cm.__exit__(None, None, None)
    nc.compile()
    return nc


# --------------------------------------------------------------------------
# public entry point
# --------------------------------------------------------------------------

_CACHED = {}


def _get_nc(cfg):
    key = cfg
    if key not in _CACHED:
        _CACHED[key] = build(cfg)
    return _CACHED[key]


def kernel(hidden_states, Wq_a, q_a_ln_w, Wq_b, Wkv_a, kv_a_ln_w, Wkv_b, Wo):
    cfg = FULL
    in_maps = prep_inputs(cfg, hidden_states, Wq_a, q_a_ln_w, Wq_b, Wkv_a,
                          kv_a_ln_w, Wkv_b, Wo)
    nc = _get_nc(cfg)
    res = run_bass_kernel_spmd(nc, in_maps, core_ids=list(range(N_CORES)))
    acc = np.zeros((cfg.D, cfg.S), np.float32)
    for r in res.results:
        acc += np.asarray(r["outT"], np.float32)
    return np.ascontiguousarray(acc.T).reshape(1, cfg.S, cfg.D)
